# revision 41
# baseline (speedup 1.0000x reference)
"""Multi-head self-attention (B=2, N=4096, C=512, H=8) on 8 trn2 NeuronCores.

Sharding: one head per core (tensor parallel over heads). Each core:
  - computes Q^T,K^T (d-on-partitions, two batches packed on partition halves)
    via one merged [wq|wk] stationary (full 128-wide PE) + a DMA redistribute,
    and V (m-on-partitions) for its head from the full token stream,
  - runs flash-style attention per batch: S^T = K Q^T via row-tiled matmul
    pairs; the softmax exp is SPLIT between the scalar (ACT) engine (exact
    exp) and the vector engine (Schraudolph bitcast exp2: one tensor_scalar
    mult+add whose int16 output bits form the bf16 of 2^(A*s+B)), so both
    engines stream the N^2 softmax concurrently; AV matmuls run several
    key-chunks behind the exp so the PE never stalls on it,
  - P^T @ [V|1] accumulated in PSUM, the appended ones-column producing the
    softmax denominators for free,
  - normalizes the head output *before* projection (vector stt multiply with
    a gpsimd-broadcast reciprocal), projects through the head's w_proj slice
    with row-packed matmul pairs, and DMA-truncates the fp32 PSUM result
    straight to bf16 DRAM partials.
Host sums the 8 bf16 partials in fp32 and adds b_proj.

Scheduling: all x DMAs are issued k-sliced up front (the 16 HW DMA engines
stream continuously from t=0); the previous block's epilogue legs (denominator
copy -> gpsimd broadcast -> reciprocal -> normalize) and its projection are
staggered one leg per key-chunk slot across the next block so the strict-FIFO
ACT/DVE queues never delay the exp stream the S^T PSUM WAR depends on.
"""

import numpy as np
import ml_dtypes

import concourse.bacc as bacc
import concourse.bass as bass
import concourse.mybir as mybir
import concourse.tile as tile
from concourse.bass_utils import run_bass_kernel_spmd

BF16 = ml_dtypes.bfloat16

B = 2
N = 4096          # sequence length per batch
C = 512           # channels
H = 8             # heads
DH = C // H       # 64 head dim
T = B * N         # total tokens
NB = 512          # query-block size
MC = 128          # key-chunk size
SCALE = float(DH) ** -0.5

# Schraudolph bitcast exp2 constants (bf16 target):
#   bf16 bits of exp(SCALE*s)  ~=  round(SCH_A*s + SCH_B)
SCH_A = 128.0 * SCALE * 1.4426950408889634
SCH_B = 16256.0 - 128.0 * 0.045

TRUNC_COMP = 1.0

N_ACT = 18        # of every 32 key chunks, this many exp'd on ACT (rest DVE)
PV_START = 8      # first chunk slot at which own-block AV drains begin
PV_DEEP = 4       # drain 2/chunk while backlog exceeds this
PV_HOLD = 31      # stop draining here; the leftover carries across the boundary
OTSB_AT = (4, 5)  # chunk slots for prev block's unnormalized head-out copies
DSB_AT = (6, 7)     # denominator row staged PSUM -> SBUF (ACT aligned move)
DMA1_AT = (8, 9)    # D row -> DRAM scratch
DMA2_AT = (10, 11)  # DRAM scratch -> [128, 4] per-partition layout
RECIP_AT = (12, 13)  # reciprocal on the tiny [128, 4] tile
PROJ_AT = (14, 16, 18, 20)
LAST_DRAIN = 26   # in the last block, drain hard from this chunk on


def _act_mask(n_act, n=32):
    # rotated so chunk 0 lands on ACT and chunk n-1 on DVE: the next block's
    # exp stream starts on the engine that is free at the boundary.
    base = [((mc + 1) * n_act) // n - (mc * n_act) // n > 0 for mc in range(n)]
    return [base[(mc + 1) % n] for mc in range(n)]


def _attention_body(nc, tc, xt, wqk, wv, wp2, out, n_seq):
    dt = mybir.dt
    cch = C // 128             # contraction chunks over C
    nblk = n_seq // NB         # query blocks per batch
    nmc = n_seq // MC          # key chunks per batch
    ntc = n_seq // 512         # 512-token chunks per batch (qkv prep)
    tpb = NB // 128            # 128-token proj chunks per query block
    EXP = mybir.ActivationFunctionType.Exp
    CPY = mybir.ActivationFunctionType.Copy
    MUL = mybir.AluOpType.mult
    ADD = mybir.AluOpType.add
    act_mask = _act_mask(N_ACT if nmc == 32 else max(1, int(round(N_ACT * nmc / 32.0))),
                         nmc)

    const = tc.alloc_tile_pool(name="const", bufs=1)
    persist = tc.alloc_tile_pool(name="persist", bufs=1)

    # constants; wqk/wv first (needed by the first prep matmuls)
    wqk_sb = const.tile([128, cch, 128], dt.bfloat16)
    nc.sync.dma_start(wqk_sb[:], wqk.rearrange("(c p) d -> p c d", p=128))
    wv_sb = const.tile([128, cch, DH], dt.bfloat16)
    nc.sync.dma_start(wv_sb[:], wv.rearrange("(c p) d -> p c d", p=128))
    wp_sb = const.tile([128, 2 * C], dt.bfloat16)  # block-diag [wp 0; 0 wp]

    # persistent per-head tensors.  qzt holds Q^T zero-padded so ONE matmul
    # computes both batches' S^T: block nb occupies cols [nb*1024,(nb+1)*1024);
    # b0 queries live on rows 0:64 x cols 0:512 of the block, b1 on rows
    # 64:128 x cols 512:1024, zeros elsewhere.
    qzt = persist.tile([128, 2 * n_seq], dt.bfloat16)
    kt2 = persist.tile([128, n_seq], dt.bfloat16)
    vext = [persist.tile([128, nmc * (DH + 1)], dt.bfloat16, name=f"vext{j}")
            for j in range(2)]
    otsb = persist.tile([128, n_seq], dt.bfloat16)  # rows 0:64 b0 head-out, 64:128 b1

    with tc.tile_pool(name="acc_ps", bufs=1, space="PSUM") as aps, \
         tc.tile_pool(name="ptp", bufs=15) as ptp, \
         tc.tile_pool(name="obp", bufs=2) as obp, \
         tc.tile_pool(name="dscrp", bufs=2, space="DRAM") as dscrp, \
         tc.tile_pool(name="rrp", bufs=2) as rrp:
      sps_cm = tc.tile_pool(name="s_ps", bufs=3, space="PSUM")
      sps = sps_cm.__enter__()
      if True:

        def emit_prep(c, xab, stgp):
            """QKV compute for one 512-token chunk of both batches (x already
            on the way via the upfront k-sliced DMAs).  PSUM comes from the
            shared S^T pool rotation (QK in cols 0:512, V tiles in 512:1024)."""
            for half in range(2):
                xa = xab[2 * c + half]
                big = sps.tile([128, 1024], dt.float32, tag="s", name="prep")
                ps = big[:, 0:512]
                psv = big[:, 512:1024].rearrange("p (t c) -> p t c", c=128)
                # merged [Q^T | K^T] for this batch: full 128-wide stationary
                for k in range(cch):
                    nc.tensor.matmul(ps, wqk_sb[:, k, :], xa[:, k, :],
                                     start=(k == 0), stop=(k == cch - 1))
                # V: [m, d] tiles, one per 128 tokens; ones column appended
                for mt in range(4):
                    for k in range(cch):
                        nc.tensor.matmul(psv[:, mt, 0:DH],
                                         xa[:, k, mt * 128:(mt + 1) * 128],
                                         wv_sb[:, k, :],
                                         start=(k == 0), stop=(k == cch - 1))
                stg = stgp.tile([128, 512], dt.bfloat16, tag="stg")
                if half == 0:
                    nc.vector.tensor_copy(stg[:], ps)
                else:
                    nc.scalar.copy(stg[:], ps)
                # redistribute to batch-packed partition halves (cross-partition)
                nc.sync.dma_start(
                    qzt[half * DH:(half + 1) * DH,
                        c * 1024 + half * 512:c * 1024 + half * 512 + 512],
                    stg[0:DH, :])
                nc.sync.dma_start(kt2[half * DH:(half + 1) * DH,
                                      c * 512:(c + 1) * 512], stg[DH:128, :])
                nc.vector.memset(psv[:, :, DH:DH + 1], 1.0)
                vdst = vext[half][:].rearrange(
                    "p (t c) -> p t c", c=DH + 1)[:, c * 4:(c + 1) * 4, :]
                if half == 0:
                    nc.vector.tensor_copy(vdst, psv[:, :, 0:DH + 1])
                else:
                    nc.scalar.copy(vdst, psv[:, :, 0:DH + 1])

        def emit_proj_gt(nb, q, rr4, jps, tail=False):
            """Projection + scaled bf16 store for the 128 tokens {4i+q} of
            block nb.  otsb holds the UNNORMALIZED head-out; the softmax
            division commutes with the projection (1/D is per-token), so it
            is applied as a per-partition scale in the PSUM->SBUF copy."""
            base = nb * NB
            ot = otsb[:, base:base + NB].rearrange("p (t s) -> p s t", s=4)
            pp = jps.tile([128, 2 * C], dt.float32, tag="s", name="pp")
            for j in range(2):
                nc.tensor.matmul(pp[:, j * C:(j + 1) * C], ot[:, q, :],
                                 wp_sb[:, j * C:(j + 1) * C], start=True,
                                 stop=True)
            for j in range(2):
                ob = obp.tile([128, C], dt.bfloat16, tag=f"ob{j}", name="ob")
                if j == 0:
                    nc.vector.tensor_scalar(ob[:], pp[:, 0:C],
                                            rr4[j][:, q:q + 1], None, MUL)
                else:
                    nc.scalar.activation(ob[:], pp[:, C:2 * C], CPY, bias=0.0,
                                         scale=rr4[j][:, q:q + 1])
                od = out[j * n_seq + base:j * n_seq + base + NB, :].rearrange(
                    "(t s) c -> s t c", s=4)
                nc.sync.dma_start(od[q], ob[:])

        nonlocal_wrm = [None]

        def emit_block(nb, prev, leftover, jps, prep_sched=None, last=False):
            wrm0 = nonlocal_wrm[0]
            """One query block's S^T/exp/AV stream.  prev is the previous
            block's (nb, acc) epilogue state; leftover is its undrained AV
            backlog, drained one chunk per early slot here (exp-independent
            PE filler for the boundary).  The epilogue legs follow at
            DSB/BCAST/RECIP/STT_AT and prev's projection at PROJ_AT.
            prep_sched maps chunk slots to prep chunks (block 0)."""
            acc = [aps.tile([DH + 1, NB], dt.float32, tag=f"acc{j}",
                            name=f"acc{j}") for j in range(2)]
            st8 = {}           # staggered epilogue state for prev
            pending = []       # own backlog: [(pt_tile, mc), ...]

            def drain_one(lst, to_acc):
                ppt, pmc = lst.pop(0)
                for j in range(2):
                    nc.tensor.matmul(
                        to_acc[j][:],
                        vext[j][:, pmc * (DH + 1):(pmc + 1) * (DH + 1)],
                        ppt[:, j * 512:j * 512 + NB],
                        start=(pmc == 0), stop=(pmc == nmc - 1))

            for mc in range(nmc):
                st = sps.tile([128, 1024], dt.float32, tag="s")
                for j in range(2):
                    nc.tensor.matmul(
                        st[:, j * 512:(j + 1) * 512],
                        kt2[:, mc * 128:(mc + 1) * 128],
                        qzt[:, nb * 1024 + j * 512:nb * 1024 + (j + 1) * 512],
                        start=True, stop=True)
                # previous block's leftover AV drains: boundary PE filler
                if leftover:
                    drain_one(leftover, prev[1])
                # staggered epilogue legs for the previous block
                if prev is not None:
                    pnb, pacc = prev
                    if mc in OTSB_AT:
                        j = mc - OTSB_AT[0]
                        if j == 0:
                            nc.scalar.copy(
                                otsb[0:DH, pnb * NB:(pnb + 1) * NB],
                                pacc[0][0:DH, :])
                        else:
                            nc.vector.tensor_copy(
                                otsb[DH:128, pnb * NB:(pnb + 1) * NB],
                                pacc[1][0:DH, :])
                    if mc in DSB_AT:
                        j = mc - DSB_AT[0]
                        dsb = rrp.tile([1, NB], dt.float32, tag=f"dsb{j}",
                                       name="dsb")
                        nc.vector.tensor_copy(dsb[:], pacc[j][DH:DH + 1, :])
                        st8[f"dsb{j}"] = dsb
                    if mc in DMA1_AT:
                        j = DMA1_AT.index(mc)
                        ds = dscrp.tile([1, NB], dt.float32, tag=f"dscr{j}",
                                        name="dscr")
                        nc.sync.dma_start(ds[:], st8[f"dsb{j}"][:])
                        st8[f"dscr{j}"] = ds
                    if mc in DMA2_AT:
                        j = DMA2_AT.index(mc)
                        d4 = rrp.tile([128, 4], dt.float32, tag=f"d4{j}",
                                      name="d4")
                        nc.sync.dma_start(
                            d4[:],
                            st8[f"dscr{j}"][:].rearrange(
                                "a (p g) -> (a p) g", p=128))
                        st8[f"d4{j}"] = d4
                    if mc in RECIP_AT:
                        j = RECIP_AT.index(mc)
                        rr4 = rrp.tile([128, 4], dt.float32, tag=f"rr4{j}",
                                       name="rr4")
                        nc.vector.reciprocal_approx_fast(rr4[:],
                                                         st8[f"d4{j}"][:])
                        st8[f"rr4{j}"] = rr4
                    if mc in PROJ_AT:
                        emit_proj_gt(pnb, PROJ_AT.index(mc),
                                     (st8["rr40"], st8["rr41"]), jps)
                pt = ptp.tile([128, 1024], dt.bfloat16, tag="pt")
                if act_mask[mc]:
                    nc.scalar.activation(pt[:], st[:], EXP, bias=0.0, scale=SCALE)
                else:
                    nc.vector.tensor_scalar(pt[:].bitcast(dt.int16), st[:],
                                            SCH_A, SCH_B, MUL, ADD)
                pending.append((pt, mc))
                if mc >= PV_START:
                    if last and mc >= LAST_DRAIN:
                        for _ in range(min(2, len(pending))):
                            drain_one(pending, acc)
                    elif mc < PV_HOLD:
                        for _ in range(min(2 if len(pending) > PV_DEEP else 1,
                                           len(pending))):
                            drain_one(pending, acc)
                if prep_sched and mc in prep_sched:
                    args = prep_sched[mc]
                    if args[0] >= 6:
                        # fake matmuls fill the unavoidable x-arrival wait so
                        # the HAM clock gate never sees a long PE idle
                        for w in range(6):
                            ps = sps.tile([128, 1024], dt.float32, tag="s",
                                          name="fill")
                            for k in range(2):
                                nc.tensor.matmul(ps[:, 0:512], wrm0[:, 0:128],
                                                 wrm0[:],
                                                 start=(k == 0), stop=(k == 1))
                    emit_prep(*args)
            if last:
                while pending:
                    drain_one(pending, acc)
            return (nb, acc), pending

        def emit_final(prev, jps):
            """Last block's epilogue + projection, minimal-latency order."""
            pnb, pacc = prev
            rr4, dss = {}, {}
            nc.scalar.copy(otsb[0:DH, pnb * NB:(pnb + 1) * NB],
                           pacc[0][0:DH, :])
            nc.vector.tensor_copy(otsb[DH:128, pnb * NB:(pnb + 1) * NB],
                                  pacc[1][0:DH, :])
            for j in range(2):
                dsb = rrp.tile([1, NB], dt.float32, tag=f"dsb{j}", name="dsb")
                nc.scalar.copy(dsb[:], pacc[j][DH:DH + 1, :])
                ds = dscrp.tile([1, NB], dt.float32, tag=f"dscr{j}", name="dscr")
                nc.sync.dma_start(ds[:], dsb[:])
                dss[j] = ds
            wrm0 = nonlocal_wrm[0]
            for w in range(10):
                # fake matmuls keep the HAM clock gate at full rate while the
                # denominator DMA round-trip completes
                ps = sps.tile([128, 1024], dt.float32, tag="s", name="tfill")
                for k in range(2):
                    nc.tensor.matmul(ps[:, 0:512], wrm0[:, 0:128], wrm0[:],
                                     start=(k == 0), stop=(k == 1))
            for j in range(2):
                d4 = rrp.tile([128, 4], dt.float32, tag=f"d4{j}", name="d4")
                nc.sync.dma_start(
                    d4[:], dss[j][:].rearrange("a (p g) -> (a p) g", p=128))
                rr4[j] = rrp.tile([128, 4], dt.float32, tag=f"rr4{j}",
                                  name="rr4")
                nc.vector.reciprocal_approx_fast(rr4[j][:], d4[:])
            for q in range(tpb):
                emit_proj_gt(pnb, q, (rr4[0], rr4[1]), jps, tail=True)

        with tc.tile_pool(name="xa", bufs=ntc) as xpool, \
             tc.tile_pool(name="stg", bufs=3) as stgp:
            # k-sliced upfront DMA issue for ALL of x in 1024-token slices
            # (2KB per partition row): the DMA engines stream continuously and
            # the first prep matmul only needs one slice.
            xtiles = []
            for cp in range(ntc // 2):
                for half in range(2):
                    xa = xpool.tile([128, cch, 1024], dt.bfloat16, tag="x")
                    base = half * n_seq + cp * 1024
                    for k in range(cch):
                        nc.scalar.dma_start(
                            xa[:, k, :],
                            xt[k * 128:(k + 1) * 128, base:base + 1024])
                    xtiles.append(xa)
                if cp == 0:
                    # zero the padding quadrants of qzt once (DVE is idle now)
                    nc.vector.memset(qzt[:], 0.0)
                if cp == 1:
                    # wp not needed until the first projection (block 1)
                    nc.sync.dma_start(wp_sb[:], wp2)
            xab = []
            for c in range(ntc):
                for half in range(2):
                    xab.append(xtiles[(c // 2) * 2 + half][
                        :, :, (c % 2) * 512:(c % 2) * 512 + 512])
            pools = (xab, stgp)
            # PE warm-up during the initial DMA wait: ~3.5us of matmuls on
            # uninitialized SBUF (no input dependency; results overwritten by
            # the first real accumulations) ramps the pstate/HAM to full rate
            # before the first prep matmul retires.
            wrm = stgp.tile([128, 512], dt.bfloat16, tag="wrm")
            nonlocal_wrm[0] = wrm
            nc.gpsimd.memset(wrm[:], 0.0)
            for w in range(8):
                ps = sps.tile([128, 1024], dt.float32, tag="s", name="warm")
                for k in range(2):
                    nc.tensor.matmul(ps[:, 0:512], wrm[:, 0:128], wrm[:],
                                     start=(k == 0), stop=(k == 1))
            for c in range(3):
                emit_prep(c, *pools)
            sched = {1 + 4 * i: (3 + i,) + pools for i in range(ntc - 3)}
            prev, leftover = emit_block(0, None, None, None, prep_sched=sched)
        for nb in range(1, nblk):
            prev, leftover = emit_block(nb, prev, leftover, sps,
                                        last=(nb == nblk - 1))
        emit_final(prev, sps)
        sps_cm.__exit__(None, None, None)

    persist.release()
    const.release()


def build_kernel(n_seq=N):
    nc = bacc.Bacc("TRN2", target_bir_lowering=False, debug=False, num_devices=8)
    dt = mybir.dt
    t_tot = 2 * n_seq
    xt = nc.dram_tensor("xt", [C, t_tot], dt.bfloat16, kind="ExternalInput").ap()
    wqk = nc.dram_tensor("wqk", [C, 128], dt.bfloat16, kind="ExternalInput").ap()
    wv = nc.dram_tensor("wv", [C, DH], dt.bfloat16, kind="ExternalInput").ap()
    wp2 = nc.dram_tensor("wp2", [128, 2 * C], dt.bfloat16,
                         kind="ExternalInput").ap()
    out = nc.dram_tensor("out", [t_tot, C], dt.bfloat16, kind="ExternalOutput").ap()
    with tile.TileContext(nc) as tc:
        _attention_body(nc, tc, xt, wqk, wv, wp2, out, n_seq)
    nc.compile()
    return nc


def make_in_maps(x, w_qkv, w_proj, n_seq=N):
    """Slice the full inputs into 8 per-core input maps (head per core)."""
    t_tot = 2 * n_seq
    xt = np.ascontiguousarray(x.reshape(t_tot, C).T).astype(BF16)
    in_maps = []
    for h in range(H):
        wq = w_qkv[h * DH:(h + 1) * DH, :].T                      # [C, DH]
        wk = w_qkv[C + h * DH:C + (h + 1) * DH, :].T
        wqk = np.ascontiguousarray(
            np.concatenate([wq, wk], axis=1)).astype(BF16)        # [C, 128]
        wv = np.ascontiguousarray(
            w_qkv[2 * C + h * DH:2 * C + (h + 1) * DH, :].T).astype(BF16)
        wp = np.ascontiguousarray(w_proj[:, h * DH:(h + 1) * DH].T)  # [DH, C]
        wp2 = np.zeros((128, 2 * C), np.float32)   # block-diag [wp 0; 0 wp]
        wp2[0:DH, 0:C] = wp
        wp2[DH:128, C:2 * C] = wp
        wp2 = wp2.astype(BF16)
        in_maps.append({"xt": xt, "wqk": wqk, "wv": wv, "wp2": wp2})
    return in_maps


_NC_CACHE = {}


def _get_nc(n_seq=N):
    if n_seq not in _NC_CACHE:
        _NC_CACHE[n_seq] = build_kernel(n_seq)
    return _NC_CACHE[n_seq]


def run(x, w_qkv, w_proj, b_proj, trace=False, tmpdir=None):
    x = np.asarray(x, dtype=np.float32)
    w_qkv = np.asarray(w_qkv, dtype=np.float32)
    w_proj = np.asarray(w_proj, dtype=np.float32)
    b_proj = np.asarray(b_proj, dtype=np.float32)
    nc = _get_nc()
    in_maps = make_in_maps(x, w_qkv, w_proj)
    try:
        res = run_bass_kernel_spmd(nc, in_maps, list(range(H)), trace=trace,
                                   tmpdir=tmpdir)
    except ModuleNotFoundError:
        res = run_bass_kernel_spmd(nc, in_maps, list(range(H)), trace=False,
                                   tmpdir=tmpdir)
    partial_sum = np.zeros((T, C), np.float32)
    for r in res.results:
        partial_sum += r["out"].astype(np.float32)
    full = partial_sum + b_proj[None, :]
    return full.reshape(B, N, C), res


def kernel(x, w_qkv, w_proj, b_proj):
    out, _ = run(x, w_qkv, w_proj, b_proj)
    return out


# revision 42
# speedup vs baseline: 1.0085x; 1.0085x over previous
"""Multi-head self-attention (B=2, N=4096, C=512, H=8) on 8 trn2 NeuronCores.

Sharding: one head per core (tensor parallel over heads). Each core:
  - computes Q^T,K^T (d-on-partitions, two batches packed on partition halves)
    via one merged [wq|wk] stationary (full 128-wide PE) + a DMA redistribute,
    and V (m-on-partitions) for its head from the full token stream,
  - runs flash-style attention per batch: S^T = K Q^T via row-tiled matmul
    pairs; the softmax exp is SPLIT between the scalar (ACT) engine (exact
    exp) and the vector engine (Schraudolph bitcast exp2: one tensor_scalar
    mult+add whose int16 output bits form the bf16 of 2^(A*s+B)), so both
    engines stream the N^2 softmax concurrently; AV matmuls run several
    key-chunks behind the exp so the PE never stalls on it,
  - P^T @ [V|1] accumulated in PSUM, the appended ones-column producing the
    softmax denominators for free,
  - normalizes the head output *before* projection (vector stt multiply with
    a gpsimd-broadcast reciprocal), projects through the head's w_proj slice
    with row-packed matmul pairs, and DMA-truncates the fp32 PSUM result
    straight to bf16 DRAM partials.
Host sums the 8 bf16 partials in fp32 and adds b_proj.

Scheduling: all x DMAs are issued k-sliced up front (the 16 HW DMA engines
stream continuously from t=0); the previous block's epilogue legs (denominator
copy -> gpsimd broadcast -> reciprocal -> normalize) and its projection are
staggered one leg per key-chunk slot across the next block so the strict-FIFO
ACT/DVE queues never delay the exp stream the S^T PSUM WAR depends on.
"""

import numpy as np
import ml_dtypes

import concourse.bacc as bacc
import concourse.bass as bass
import concourse.mybir as mybir
import concourse.tile as tile
from concourse.bass_utils import run_bass_kernel_spmd

BF16 = ml_dtypes.bfloat16

B = 2
N = 4096          # sequence length per batch
C = 512           # channels
H = 8             # heads
DH = C // H       # 64 head dim
T = B * N         # total tokens
NB = 512          # query-block size
MC = 128          # key-chunk size
SCALE = float(DH) ** -0.5

# Schraudolph bitcast exp2 constants (bf16 target):
#   bf16 bits of exp(SCALE*s)  ~=  round(SCH_A*s + SCH_B)
SCH_A = 128.0 * SCALE * 1.4426950408889634
SCH_B = 16256.0 - 128.0 * 0.045

TRUNC_COMP = 1.0

N_ACT = 18        # of every 32 key chunks, this many exp'd on ACT (rest DVE)
PV_START = 8      # first chunk slot at which own-block AV drains begin
PV_DEEP = 4       # drain 2/chunk while backlog exceeds this
PV_HOLD = 31      # stop draining here; the leftover carries across the boundary
OTSB_AT = (4, 5)  # chunk slots for prev block's unnormalized head-out copies
DSB_AT = (5, 6)     # denominator row staged PSUM -> SBUF (ACT aligned move)
RECIP_AT = (7, 9)   # reciprocal of the D row
DMA1_AT = (9, 11)   # reciprocal row -> DRAM scratch
DMA2_AT = (11, 13)  # DRAM scratch -> [128, 4] per-partition layout
PROJ_AT = (14, 16, 18, 20)
LAST_DRAIN = 26   # in the last block, drain hard from this chunk on


def _act_mask(n_act, n=32):
    # rotated so chunk 0 lands on ACT and chunk n-1 on DVE: the next block's
    # exp stream starts on the engine that is free at the boundary.
    base = [((mc + 1) * n_act) // n - (mc * n_act) // n > 0 for mc in range(n)]
    return [base[(mc + 1) % n] for mc in range(n)]


def _attention_body(nc, tc, xt, wqk, wv, wp2, out, n_seq):
    dt = mybir.dt
    cch = C // 128             # contraction chunks over C
    nblk = n_seq // NB         # query blocks per batch
    nmc = n_seq // MC          # key chunks per batch
    ntc = n_seq // 512         # 512-token chunks per batch (qkv prep)
    tpb = NB // 128            # 128-token proj chunks per query block
    EXP = mybir.ActivationFunctionType.Exp
    CPY = mybir.ActivationFunctionType.Copy
    MUL = mybir.AluOpType.mult
    ADD = mybir.AluOpType.add
    act_mask = _act_mask(N_ACT if nmc == 32 else max(1, int(round(N_ACT * nmc / 32.0))),
                         nmc)

    const = tc.alloc_tile_pool(name="const", bufs=1)
    persist = tc.alloc_tile_pool(name="persist", bufs=1)

    # constants; wqk/wv first (needed by the first prep matmuls)
    wqk_sb = const.tile([128, cch, 128], dt.bfloat16)
    nc.sync.dma_start(wqk_sb[:], wqk.rearrange("(c p) d -> p c d", p=128))
    wv_sb = const.tile([128, cch, DH], dt.bfloat16)
    nc.sync.dma_start(wv_sb[:], wv.rearrange("(c p) d -> p c d", p=128))
    wp_sb = const.tile([128, 2 * C], dt.bfloat16)  # block-diag [wp 0; 0 wp]

    # persistent per-head tensors.  qzt holds Q^T zero-padded so ONE matmul
    # computes both batches' S^T: block nb occupies cols [nb*1024,(nb+1)*1024);
    # b0 queries live on rows 0:64 x cols 0:512 of the block, b1 on rows
    # 64:128 x cols 512:1024, zeros elsewhere.
    qzt = persist.tile([128, 2 * n_seq], dt.bfloat16)
    kt2 = persist.tile([128, n_seq], dt.bfloat16)
    vext = [persist.tile([128, nmc * (DH + 1)], dt.bfloat16, name=f"vext{j}")
            for j in range(2)]
    otsb = persist.tile([128, n_seq], dt.bfloat16)  # rows 0:64 b0 head-out, 64:128 b1

    with tc.tile_pool(name="acc_ps", bufs=1, space="PSUM") as aps, \
         tc.tile_pool(name="ptp", bufs=15) as ptp, \
         tc.tile_pool(name="obp", bufs=2) as obp, \
         tc.tile_pool(name="dscrp", bufs=2, space="DRAM") as dscrp, \
         tc.tile_pool(name="rrp", bufs=2) as rrp:
      sps_cm = tc.tile_pool(name="s_ps", bufs=3, space="PSUM")
      sps = sps_cm.__enter__()
      if True:

        def emit_prep(c, xab, stgp):
            """QKV compute for one 512-token chunk of both batches (x already
            on the way via the upfront k-sliced DMAs).  PSUM comes from the
            shared S^T pool rotation (QK in cols 0:512, V tiles in 512:1024)."""
            for half in range(2):
                xa = xab[2 * c + half]
                big = sps.tile([128, 1024], dt.float32, tag="s", name="prep")
                ps = big[:, 0:512]
                psv = big[:, 512:1024].rearrange("p (t c) -> p t c", c=128)
                # merged [Q^T | K^T] for this batch: full 128-wide stationary
                for k in range(cch):
                    nc.tensor.matmul(ps, wqk_sb[:, k, :], xa[:, k, :],
                                     start=(k == 0), stop=(k == cch - 1))
                # V: [m, d] tiles, one per 128 tokens; ones column appended
                for mt in range(4):
                    for k in range(cch):
                        nc.tensor.matmul(psv[:, mt, 0:DH],
                                         xa[:, k, mt * 128:(mt + 1) * 128],
                                         wv_sb[:, k, :],
                                         start=(k == 0), stop=(k == cch - 1))
                stg = stgp.tile([128, 512], dt.bfloat16, tag="stg")
                if half == 0:
                    nc.vector.tensor_copy(stg[:], ps)
                else:
                    nc.scalar.copy(stg[:], ps)
                # redistribute to batch-packed partition halves (cross-partition)
                nc.sync.dma_start(
                    qzt[half * DH:(half + 1) * DH,
                        c * 1024 + half * 512:c * 1024 + half * 512 + 512],
                    stg[0:DH, :])
                nc.sync.dma_start(kt2[half * DH:(half + 1) * DH,
                                      c * 512:(c + 1) * 512], stg[DH:128, :])
                nc.vector.memset(psv[:, :, DH:DH + 1], 1.0)
                vdst = vext[half][:].rearrange(
                    "p (t c) -> p t c", c=DH + 1)[:, c * 4:(c + 1) * 4, :]
                if half == 0:
                    nc.vector.tensor_copy(vdst, psv[:, :, 0:DH + 1])
                else:
                    nc.scalar.copy(vdst, psv[:, :, 0:DH + 1])

        def emit_proj_gt(nb, q, rr4, jps, tail=False):
            """Projection + scaled bf16 store for the 128 tokens {4i+q} of
            block nb.  otsb holds the UNNORMALIZED head-out; the softmax
            division commutes with the projection (1/D is per-token), so it
            is applied as a per-partition scale in the PSUM->SBUF copy."""
            base = nb * NB
            ot = otsb[:, base:base + NB].rearrange("p (t s) -> p s t", s=4)
            pp = jps.tile([128, 2 * C], dt.float32, tag="s", name="pp")
            for j in range(2):
                nc.tensor.matmul(pp[:, j * C:(j + 1) * C], ot[:, q, :],
                                 wp_sb[:, j * C:(j + 1) * C], start=True,
                                 stop=True)
            for j in range(2):
                ob = obp.tile([128, C], dt.bfloat16, tag=f"ob{j}", name="ob")
                if j == 0:
                    nc.vector.tensor_scalar(ob[:], pp[:, 0:C],
                                            rr4[j][:, q:q + 1], None, MUL)
                else:
                    nc.scalar.activation(ob[:], pp[:, C:2 * C], CPY, bias=0.0,
                                         scale=rr4[j][:, q:q + 1])
                od = out[j * n_seq + base:j * n_seq + base + NB, :].rearrange(
                    "(t s) c -> s t c", s=4)
                nc.sync.dma_start(od[q], ob[:])

        nonlocal_wrm = [None]

        def emit_block(nb, prev, leftover, jps, prep_sched=None, last=False):
            wrm0 = nonlocal_wrm[0]
            """One query block's S^T/exp/AV stream.  prev is the previous
            block's (nb, acc) epilogue state; leftover is its undrained AV
            backlog, drained one chunk per early slot here (exp-independent
            PE filler for the boundary).  The epilogue legs follow at
            DSB/BCAST/RECIP/STT_AT and prev's projection at PROJ_AT.
            prep_sched maps chunk slots to prep chunks (block 0)."""
            acc = [aps.tile([DH + 1, NB], dt.float32, tag=f"acc{j}",
                            name=f"acc{j}") for j in range(2)]
            st8 = {}           # staggered epilogue state for prev
            pending = []       # own backlog: [(pt_tile, mc), ...]

            def drain_one(lst, to_acc):
                ppt, pmc = lst.pop(0)
                for j in range(2):
                    nc.tensor.matmul(
                        to_acc[j][:],
                        vext[j][:, pmc * (DH + 1):(pmc + 1) * (DH + 1)],
                        ppt[:, j * 512:j * 512 + NB],
                        start=(pmc == 0), stop=(pmc == nmc - 1))

            for mc in range(nmc):
                st = sps.tile([128, 1024], dt.float32, tag="s")
                for j in range(2):
                    nc.tensor.matmul(
                        st[:, j * 512:(j + 1) * 512],
                        kt2[:, mc * 128:(mc + 1) * 128],
                        qzt[:, nb * 1024 + j * 512:nb * 1024 + (j + 1) * 512],
                        start=True, stop=True)
                # previous block's leftover AV drains: boundary PE filler
                if leftover:
                    drain_one(leftover, prev[1])
                # staggered epilogue legs for the previous block
                if prev is not None:
                    pnb, pacc = prev
                    if mc in OTSB_AT:
                        j = mc - OTSB_AT[0]
                        if j == 0:
                            nc.scalar.copy(
                                otsb[0:DH, pnb * NB:(pnb + 1) * NB],
                                pacc[0][0:DH, :])
                        else:
                            nc.vector.tensor_copy(
                                otsb[DH:128, pnb * NB:(pnb + 1) * NB],
                                pacc[1][0:DH, :])
                    if mc in DSB_AT:
                        j = mc - DSB_AT[0]
                        dsb = rrp.tile([1, NB], dt.float32, tag=f"dsb{j}",
                                       name="dsb")
                        nc.scalar.copy(dsb[:], pacc[j][DH:DH + 1, :])
                        st8[f"dsb{j}"] = dsb
                    if mc in RECIP_AT:
                        j = RECIP_AT.index(mc)
                        rinv = rrp.tile([1, NB], dt.float32, tag=f"rinv{j}",
                                        name="rinv")
                        nc.vector.reciprocal_approx_fast(rinv[:],
                                                         st8[f"dsb{j}"][:])
                        st8[f"rinv{j}"] = rinv
                    if mc in DMA1_AT:
                        j = DMA1_AT.index(mc)
                        ds = dscrp.tile([1, NB], dt.float32, tag=f"dscr{j}",
                                        name="dscr")
                        nc.sync.dma_start(ds[:], st8[f"rinv{j}"][:])
                        st8[f"dscr{j}"] = ds
                    if mc in DMA2_AT:
                        j = DMA2_AT.index(mc)
                        rr4 = rrp.tile([128, 4], dt.float32, tag=f"rr4{j}",
                                       name="rr4")
                        nc.sync.dma_start(
                            rr4[:],
                            st8[f"dscr{j}"][:].rearrange(
                                "a (p g) -> (a p) g", p=128))
                        st8[f"rr4{j}"] = rr4
                    if mc in PROJ_AT:
                        emit_proj_gt(pnb, PROJ_AT.index(mc),
                                     (st8["rr40"], st8["rr41"]), jps)
                pt = ptp.tile([128, 1024], dt.bfloat16, tag="pt")
                if act_mask[mc]:
                    nc.scalar.activation(pt[:], st[:], EXP, bias=0.0, scale=SCALE)
                else:
                    nc.vector.tensor_scalar(pt[:].bitcast(dt.int16), st[:],
                                            SCH_A, SCH_B, MUL, ADD)
                pending.append((pt, mc))
                if mc >= PV_START:
                    if last and mc >= LAST_DRAIN:
                        for _ in range(min(2, len(pending))):
                            drain_one(pending, acc)
                    elif mc < PV_HOLD:
                        for _ in range(min(2 if len(pending) > PV_DEEP else 1,
                                           len(pending))):
                            drain_one(pending, acc)
                if prep_sched and mc in prep_sched:
                    args = prep_sched[mc]
                    if args[0] >= 6:
                        # fake matmuls fill the unavoidable x-arrival wait so
                        # the HAM clock gate never sees a long PE idle
                        for w in range(6):
                            ps = sps.tile([128, 1024], dt.float32, tag="s",
                                          name="fill")
                            for k in range(2):
                                nc.tensor.matmul(ps[:, 0:512], wrm0[:, 0:128],
                                                 wrm0[:],
                                                 start=(k == 0), stop=(k == 1))
                    emit_prep(*args)
            if last:
                while pending:
                    drain_one(pending, acc)
            return (nb, acc), pending

        def emit_final(prev, jps):
            """Last block's epilogue + projection, minimal-latency order."""
            pnb, pacc = prev
            rr4, dss = {}, {}
            for j in range(2):
                dsb = rrp.tile([1, NB], dt.float32, tag=f"dsb{j}", name="dsb")
                nc.scalar.copy(dsb[:], pacc[j][DH:DH + 1, :])
                rinv = rrp.tile([1, NB], dt.float32, tag=f"rinv{j}",
                                name="rinv")
                nc.vector.reciprocal_approx_fast(rinv[:], dsb[:])
                ds = dscrp.tile([1, NB], dt.float32, tag=f"dscr{j}", name="dscr")
                nc.sync.dma_start(ds[:], rinv[:])
                dss[j] = ds
            nc.scalar.copy(otsb[0:DH, pnb * NB:(pnb + 1) * NB],
                           pacc[0][0:DH, :])
            nc.vector.tensor_copy(otsb[DH:128, pnb * NB:(pnb + 1) * NB],
                                  pacc[1][0:DH, :])
            wrm0 = nonlocal_wrm[0]
            for w in range(6):
                # fake matmuls keep the HAM clock gate at full rate while the
                # denominator DMA round-trip completes
                ps = sps.tile([128, 1024], dt.float32, tag="s", name="tfill")
                for k in range(2):
                    nc.tensor.matmul(ps[:, 0:512], wrm0[:, 0:128], wrm0[:],
                                     start=(k == 0), stop=(k == 1))
            for j in range(2):
                rr4[j] = rrp.tile([128, 4], dt.float32, tag=f"rr4{j}",
                                  name="rr4")
                nc.sync.dma_start(
                    rr4[j][:],
                    dss[j][:].rearrange("a (p g) -> (a p) g", p=128))
            for q in range(tpb):
                emit_proj_gt(pnb, q, (rr4[0], rr4[1]), jps, tail=True)

        with tc.tile_pool(name="xa", bufs=ntc) as xpool, \
             tc.tile_pool(name="stg", bufs=3) as stgp:
            # k-sliced upfront DMA issue for ALL of x in 1024-token slices
            # (2KB per partition row): the DMA engines stream continuously and
            # the first prep matmul only needs one slice.
            xtiles = []
            for cp in range(ntc // 2):
                for half in range(2):
                    xa = xpool.tile([128, cch, 1024], dt.bfloat16, tag="x")
                    base = half * n_seq + cp * 1024
                    for k in range(cch):
                        nc.scalar.dma_start(
                            xa[:, k, :],
                            xt[k * 128:(k + 1) * 128, base:base + 1024])
                    xtiles.append(xa)
                if cp == 0:
                    # zero the padding quadrants of qzt once (DVE is idle now)
                    nc.vector.memset(qzt[:], 0.0)
                if cp == 1:
                    # wp not needed until the first projection (block 1)
                    nc.sync.dma_start(wp_sb[:], wp2)
            xab = []
            for c in range(ntc):
                for half in range(2):
                    xab.append(xtiles[(c // 2) * 2 + half][
                        :, :, (c % 2) * 512:(c % 2) * 512 + 512])
            pools = (xab, stgp)
            # PE warm-up during the initial DMA wait: ~3.5us of matmuls on
            # uninitialized SBUF (no input dependency; results overwritten by
            # the first real accumulations) ramps the pstate/HAM to full rate
            # before the first prep matmul retires.
            wrm = stgp.tile([128, 512], dt.bfloat16, tag="wrm")
            nonlocal_wrm[0] = wrm
            nc.gpsimd.memset(wrm[:], 0.0)
            for w in range(8):
                ps = sps.tile([128, 1024], dt.float32, tag="s", name="warm")
                for k in range(2):
                    nc.tensor.matmul(ps[:, 0:512], wrm[:, 0:128], wrm[:],
                                     start=(k == 0), stop=(k == 1))
            for c in range(3):
                emit_prep(c, *pools)
            sched = {1 + 4 * i: (3 + i,) + pools for i in range(ntc - 3)}
            prev, leftover = emit_block(0, None, None, None, prep_sched=sched)
        for nb in range(1, nblk):
            prev, leftover = emit_block(nb, prev, leftover, sps,
                                        last=(nb == nblk - 1))
        emit_final(prev, sps)
        sps_cm.__exit__(None, None, None)

    persist.release()
    const.release()


def build_kernel(n_seq=N):
    nc = bacc.Bacc("TRN2", target_bir_lowering=False, debug=False, num_devices=8)
    dt = mybir.dt
    t_tot = 2 * n_seq
    xt = nc.dram_tensor("xt", [C, t_tot], dt.bfloat16, kind="ExternalInput").ap()
    wqk = nc.dram_tensor("wqk", [C, 128], dt.bfloat16, kind="ExternalInput").ap()
    wv = nc.dram_tensor("wv", [C, DH], dt.bfloat16, kind="ExternalInput").ap()
    wp2 = nc.dram_tensor("wp2", [128, 2 * C], dt.bfloat16,
                         kind="ExternalInput").ap()
    out = nc.dram_tensor("out", [t_tot, C], dt.bfloat16, kind="ExternalOutput").ap()
    with tile.TileContext(nc) as tc:
        _attention_body(nc, tc, xt, wqk, wv, wp2, out, n_seq)
    nc.compile()
    return nc


def make_in_maps(x, w_qkv, w_proj, n_seq=N):
    """Slice the full inputs into 8 per-core input maps (head per core)."""
    t_tot = 2 * n_seq
    xt = np.ascontiguousarray(x.reshape(t_tot, C).T).astype(BF16)
    in_maps = []
    for h in range(H):
        wq = w_qkv[h * DH:(h + 1) * DH, :].T                      # [C, DH]
        wk = w_qkv[C + h * DH:C + (h + 1) * DH, :].T
        wqk = np.ascontiguousarray(
            np.concatenate([wq, wk], axis=1)).astype(BF16)        # [C, 128]
        wv = np.ascontiguousarray(
            w_qkv[2 * C + h * DH:2 * C + (h + 1) * DH, :].T).astype(BF16)
        wp = np.ascontiguousarray(w_proj[:, h * DH:(h + 1) * DH].T)  # [DH, C]
        wp2 = np.zeros((128, 2 * C), np.float32)   # block-diag [wp 0; 0 wp]
        wp2[0:DH, 0:C] = wp
        wp2[DH:128, C:2 * C] = wp
        wp2 = wp2.astype(BF16)
        in_maps.append({"xt": xt, "wqk": wqk, "wv": wv, "wp2": wp2})
    return in_maps


_NC_CACHE = {}


def _get_nc(n_seq=N):
    if n_seq not in _NC_CACHE:
        _NC_CACHE[n_seq] = build_kernel(n_seq)
    return _NC_CACHE[n_seq]


def run(x, w_qkv, w_proj, b_proj, trace=False, tmpdir=None):
    x = np.asarray(x, dtype=np.float32)
    w_qkv = np.asarray(w_qkv, dtype=np.float32)
    w_proj = np.asarray(w_proj, dtype=np.float32)
    b_proj = np.asarray(b_proj, dtype=np.float32)
    nc = _get_nc()
    in_maps = make_in_maps(x, w_qkv, w_proj)
    try:
        res = run_bass_kernel_spmd(nc, in_maps, list(range(H)), trace=trace,
                                   tmpdir=tmpdir)
    except ModuleNotFoundError:
        res = run_bass_kernel_spmd(nc, in_maps, list(range(H)), trace=False,
                                   tmpdir=tmpdir)
    partial_sum = np.zeros((T, C), np.float32)
    for r in res.results:
        partial_sum += r["out"].astype(np.float32)
    full = partial_sum + b_proj[None, :]
    return full.reshape(B, N, C), res


def kernel(x, w_qkv, w_proj, b_proj):
    out, _ = run(x, w_qkv, w_proj, b_proj)
    return out


# revision 43
# speedup vs baseline: 1.0381x; 1.0293x over previous
"""Multi-head self-attention (B=2, N=4096, C=512, H=8) on 8 trn2 NeuronCores.

Sharding: one head per core (tensor parallel over heads). Each core:
  - computes Q^T,K^T (d-on-partitions, two batches packed on partition halves)
    via one merged [wq|wk] stationary (full 128-wide PE) + a DMA redistribute,
    and V (m-on-partitions) for its head from the full token stream,
  - runs flash-style attention per batch: S^T = K Q^T via row-tiled matmul
    pairs; the softmax exp is SPLIT between the scalar (ACT) engine (exact
    exp) and the vector engine (Schraudolph bitcast exp2: one tensor_scalar
    mult+add whose int16 output bits form the bf16 of 2^(A*s+B)), so both
    engines stream the N^2 softmax concurrently; AV matmuls run several
    key-chunks behind the exp so the PE never stalls on it,
  - P^T @ [V|1] accumulated in PSUM, the appended ones-column producing the
    softmax denominators for free,
  - normalizes the head output *before* projection (vector stt multiply with
    a gpsimd-broadcast reciprocal), projects through the head's w_proj slice
    with row-packed matmul pairs, and DMA-truncates the fp32 PSUM result
    straight to bf16 DRAM partials.
Host sums the 8 bf16 partials in fp32 and adds b_proj.

Scheduling: all x DMAs are issued k-sliced up front (the 16 HW DMA engines
stream continuously from t=0); the previous block's epilogue legs (denominator
copy -> gpsimd broadcast -> reciprocal -> normalize) and its projection are
staggered one leg per key-chunk slot across the next block so the strict-FIFO
ACT/DVE queues never delay the exp stream the S^T PSUM WAR depends on.
"""

import numpy as np
import ml_dtypes

import concourse.bacc as bacc
import concourse.bass as bass
import concourse.mybir as mybir
import concourse.tile as tile
from concourse.bass_utils import run_bass_kernel_spmd

BF16 = ml_dtypes.bfloat16

B = 2
N = 4096          # sequence length per batch
C = 512           # channels
H = 8             # heads
DH = C // H       # 64 head dim
T = B * N         # total tokens
NB = 512          # query-block size
MC = 128          # key-chunk size
SCALE = float(DH) ** -0.5

# Schraudolph bitcast exp2 constants (bf16 target):
#   bf16 bits of exp(SCALE*s)  ~=  round(SCH_A*s + SCH_B)
SCH_A = 128.0 * SCALE * 1.4426950408889634
SCH_B = 16256.0 - 128.0 * 0.045

TRUNC_COMP = 1.0

N_ACT = 18        # of every 32 key chunks, this many exp'd on ACT (rest DVE)
PV_START = 8      # first chunk slot at which own-block AV drains begin
PV_DEEP = 4       # drain 2/chunk while backlog exceeds this
PV_HOLD = 31      # stop draining here; the leftover carries across the boundary
OTSB_AT = (4, 5)  # chunk slots for prev block's unnormalized head-out copies
DSB_AT = (5, 6)     # denominator row staged PSUM -> SBUF (ACT aligned move)
RECIP_AT = (7, 9)   # reciprocal of the D row
DMA1_AT = (9, 11)   # reciprocal row -> DRAM scratch
DMA2_AT = (11, 13)  # DRAM scratch -> [128, 4] per-partition layout
PROJ_AT = (14, 16, 18, 20)
LAST_DRAIN = 26   # in the last block, drain hard from this chunk on


def _act_mask(n_act, n=32):
    # rotated so chunk 0 lands on ACT and chunk n-1 on DVE: the next block's
    # exp stream starts on the engine that is free at the boundary.
    base = [((mc + 1) * n_act) // n - (mc * n_act) // n > 0 for mc in range(n)]
    return [base[(mc + 1) % n] for mc in range(n)]


def _attention_body(nc, tc, xt, wqk, wv, wp2, out, n_seq):
    dt = mybir.dt
    cch = C // 128             # contraction chunks over C
    nblk = n_seq // NB         # query blocks per batch
    nmc = n_seq // MC          # key chunks per batch
    ntc = n_seq // 512         # 512-token chunks per batch (qkv prep)
    tpb = NB // 128            # 128-token proj chunks per query block
    EXP = mybir.ActivationFunctionType.Exp
    CPY = mybir.ActivationFunctionType.Copy
    MUL = mybir.AluOpType.mult
    ADD = mybir.AluOpType.add
    act_mask = _act_mask(N_ACT if nmc == 32 else max(1, int(round(N_ACT * nmc / 32.0))),
                         nmc)

    const = tc.alloc_tile_pool(name="const", bufs=1)
    persist = tc.alloc_tile_pool(name="persist", bufs=1)

    # constants; wqk/wv first (needed by the first prep matmuls)
    wqk_sb = const.tile([128, cch, 128], dt.bfloat16)
    nc.sync.dma_start(wqk_sb[:], wqk.rearrange("(c p) d -> p c d", p=128))
    wv_sb = const.tile([128, cch, DH], dt.bfloat16)
    nc.sync.dma_start(wv_sb[:], wv.rearrange("(c p) d -> p c d", p=128))
    wp_sb = const.tile([128, 2 * C], dt.bfloat16)  # block-diag [wp 0; 0 wp]

    # persistent per-head tensors.  qzt holds Q^T zero-padded so ONE matmul
    # computes both batches' S^T: block nb occupies cols [nb*1024,(nb+1)*1024);
    # b0 queries live on rows 0:64 x cols 0:512 of the block, b1 on rows
    # 64:128 x cols 512:1024, zeros elsewhere.
    qzt = persist.tile([128, 2 * n_seq], dt.bfloat16)
    kt2 = persist.tile([128, n_seq], dt.bfloat16)
    vext = [persist.tile([128, nmc * (DH + 1)], dt.bfloat16, name=f"vext{j}")
            for j in range(2)]
    otsb = persist.tile([128, n_seq], dt.bfloat16)  # rows 0:64 b0 head-out, 64:128 b1

    with tc.tile_pool(name="acc_ps", bufs=1, space="PSUM") as aps, \
         tc.tile_pool(name="ptp", bufs=15) as ptp, \
         tc.tile_pool(name="obp", bufs=2) as obp, \
         tc.tile_pool(name="dscrp", bufs=2, space="DRAM") as dscrp, \
         tc.tile_pool(name="rrp", bufs=2) as rrp:
      sps_cm = tc.tile_pool(name="s_ps", bufs=3, space="PSUM")
      sps = sps_cm.__enter__()
      if True:

        def emit_prep(c, xab, stgp):
            """QKV compute for one 512-token chunk of both batches (x already
            on the way via the upfront k-sliced DMAs).  PSUM comes from the
            shared S^T pool rotation (QK in cols 0:512, V tiles in 512:1024)."""
            for half in range(2):
                xa = xab[2 * c + half]
                big = sps.tile([128, 1024], dt.float32, tag="s", name="prep")
                ps = big[:, 0:512]
                psv = big[:, 512:1024].rearrange("p (t c) -> p t c", c=128)
                # merged [Q^T | K^T] for this batch: full 128-wide stationary
                for k in range(cch):
                    nc.tensor.matmul(ps, wqk_sb[:, k, :], xa[:, k, :],
                                     start=(k == 0), stop=(k == cch - 1))
                # V: [m, d] tiles, one per 128 tokens; ones column appended
                for mt in range(4):
                    for k in range(cch):
                        nc.tensor.matmul(psv[:, mt, 0:DH],
                                         xa[:, k, mt * 128:(mt + 1) * 128],
                                         wv_sb[:, k, :],
                                         start=(k == 0), stop=(k == cch - 1))
                stg = stgp.tile([128, 512], dt.bfloat16, tag="stg")
                if half == 0:
                    nc.vector.tensor_copy(stg[:], ps)
                else:
                    nc.scalar.copy(stg[:], ps)
                # redistribute to batch-packed partition halves (cross-partition)
                nc.sync.dma_start(
                    qzt[half * DH:(half + 1) * DH,
                        c * 1024 + half * 512:c * 1024 + half * 512 + 512],
                    stg[0:DH, :])
                nc.sync.dma_start(kt2[half * DH:(half + 1) * DH,
                                      c * 512:(c + 1) * 512], stg[DH:128, :])
                nc.vector.memset(psv[:, :, DH:DH + 1], 1.0)
                vdst = vext[half][:].rearrange(
                    "p (t c) -> p t c", c=DH + 1)[:, c * 4:(c + 1) * 4, :]
                if half == 0:
                    nc.vector.tensor_copy(vdst, psv[:, :, 0:DH + 1])
                else:
                    nc.scalar.copy(vdst, psv[:, :, 0:DH + 1])

        def emit_proj_gt(nb, q, rr4, jps, tail=False):
            """Projection + scaled bf16 store for the 128 tokens {4i+q} of
            block nb.  otsb holds the UNNORMALIZED head-out; the softmax
            division commutes with the projection (1/D is per-token), so it
            is applied as a per-partition scale in the PSUM->SBUF copy."""
            base = nb * NB
            ot = otsb[:, base:base + NB].rearrange("p (t s) -> p s t", s=4)
            pp = jps.tile([128, 2 * C], dt.float32, tag="s", name="pp")
            for j in range(2):
                nc.tensor.matmul(pp[:, j * C:(j + 1) * C], ot[:, q, :],
                                 wp_sb[:, j * C:(j + 1) * C], start=True,
                                 stop=True)
            for j in range(2):
                ob = obp.tile([128, C], dt.bfloat16, tag=f"ob{j}", name="ob")
                if j == 0:
                    nc.vector.tensor_scalar(ob[:], pp[:, 0:C],
                                            rr4[j][:, q:q + 1], None, MUL)
                else:
                    nc.scalar.activation(ob[:], pp[:, C:2 * C], CPY, bias=0.0,
                                         scale=rr4[j][:, q:q + 1])
                od = out[j * n_seq + base:j * n_seq + base + NB, :].rearrange(
                    "(t s) c -> s t c", s=4)
                nc.sync.dma_start(od[q], ob[:])

        nonlocal_wrm = [None]

        def emit_block(nb, prev, leftover, jps, prep_sched=None, last=False):
            wrm0 = nonlocal_wrm[0]
            """One query block's S^T/exp/AV stream.  prev is the previous
            block's (nb, acc) epilogue state; leftover is its undrained AV
            backlog, drained one chunk per early slot here (exp-independent
            PE filler for the boundary).  The epilogue legs follow at
            DSB/BCAST/RECIP/STT_AT and prev's projection at PROJ_AT.
            prep_sched maps chunk slots to prep chunks (block 0)."""
            acc = [aps.tile([DH + 1, NB], dt.float32, tag=f"acc{j}",
                            name=f"acc{j}") for j in range(2)]
            st8 = {}           # staggered epilogue state for prev
            pending = []       # own backlog: [(pt_tile, mc), ...]

            def drain_one(lst, to_acc):
                ppt, pmc = lst.pop(0)
                for j in range(2):
                    nc.tensor.matmul(
                        to_acc[j][:],
                        vext[j][:, pmc * (DH + 1):(pmc + 1) * (DH + 1)],
                        ppt[:, j * 512:j * 512 + NB],
                        start=(pmc == 0), stop=(pmc == nmc - 1))

            for mc in range(nmc):
                st = sps.tile([128, 1024], dt.float32, tag="s")
                for j in range(2):
                    nc.tensor.matmul(
                        st[:, j * 512:(j + 1) * 512],
                        kt2[:, mc * 128:(mc + 1) * 128],
                        qzt[:, nb * 1024 + j * 512:nb * 1024 + (j + 1) * 512],
                        start=True, stop=True)
                # previous block's leftover AV drains: boundary PE filler
                if leftover:
                    drain_one(leftover, prev[1])
                # staggered epilogue legs for the previous block
                if prev is not None:
                    pnb, pacc = prev
                    if mc in OTSB_AT:
                        j = mc - OTSB_AT[0]
                        if j == 0:
                            nc.scalar.copy(
                                otsb[0:DH, pnb * NB:(pnb + 1) * NB],
                                pacc[0][0:DH, :])
                        else:
                            nc.vector.tensor_copy(
                                otsb[DH:128, pnb * NB:(pnb + 1) * NB],
                                pacc[1][0:DH, :])
                    if mc in DSB_AT:
                        j = mc - DSB_AT[0]
                        dsb = rrp.tile([1, NB], dt.float32, tag=f"dsb{j}",
                                       name="dsb")
                        nc.scalar.copy(dsb[:], pacc[j][DH:DH + 1, :])
                        st8[f"dsb{j}"] = dsb
                    if mc in RECIP_AT:
                        j = RECIP_AT.index(mc)
                        rinv = rrp.tile([1, NB], dt.float32, tag=f"rinv{j}",
                                        name="rinv")
                        nc.vector.reciprocal_approx_fast(rinv[:],
                                                         st8[f"dsb{j}"][:])
                        st8[f"rinv{j}"] = rinv
                    if mc in DMA1_AT:
                        j = DMA1_AT.index(mc)
                        ds = dscrp.tile([1, NB], dt.float32, tag=f"dscr{j}",
                                        name="dscr")
                        nc.sync.dma_start(ds[:], st8[f"rinv{j}"][:])
                        st8[f"dscr{j}"] = ds
                    if mc in DMA2_AT:
                        j = DMA2_AT.index(mc)
                        rr4 = rrp.tile([128, 4], dt.float32, tag=f"rr4{j}",
                                       name="rr4")
                        nc.sync.dma_start(
                            rr4[:],
                            st8[f"dscr{j}"][:].rearrange(
                                "a (p g) -> (a p) g", p=128))
                        st8[f"rr4{j}"] = rr4
                    if mc in PROJ_AT:
                        emit_proj_gt(pnb, PROJ_AT.index(mc),
                                     (st8["rr40"], st8["rr41"]), jps)
                pt = ptp.tile([128, 1024], dt.bfloat16, tag="pt")
                if act_mask[mc]:
                    nc.scalar.activation(pt[:], st[:], EXP, bias=0.0, scale=SCALE)
                else:
                    nc.vector.tensor_scalar(pt[:].bitcast(dt.int16), st[:],
                                            SCH_A, SCH_B, MUL, ADD)
                pending.append((pt, mc))
                if mc >= PV_START:
                    if last and mc >= LAST_DRAIN:
                        for _ in range(min(2, len(pending))):
                            drain_one(pending, acc)
                    elif mc < PV_HOLD:
                        for _ in range(min(2 if len(pending) > PV_DEEP else 1,
                                           len(pending))):
                            drain_one(pending, acc)
                if prep_sched and mc in prep_sched:
                    emit_prep(*prep_sched[mc])
            if last:
                while pending:
                    drain_one(pending, acc)
            return (nb, acc), pending

        def emit_final(prev, jps):
            """Last block's epilogue + projection, minimal-latency order."""
            pnb, pacc = prev
            rr4, dss = {}, {}
            for j in range(2):
                dsb = rrp.tile([1, NB], dt.float32, tag=f"dsb{j}", name="dsb")
                nc.scalar.copy(dsb[:], pacc[j][DH:DH + 1, :])
                rinv = rrp.tile([1, NB], dt.float32, tag=f"rinv{j}",
                                name="rinv")
                nc.vector.reciprocal_approx_fast(rinv[:], dsb[:])
                ds = dscrp.tile([1, NB], dt.float32, tag=f"dscr{j}", name="dscr")
                nc.sync.dma_start(ds[:], rinv[:])
                dss[j] = ds
            nc.scalar.copy(otsb[0:DH, pnb * NB:(pnb + 1) * NB],
                           pacc[0][0:DH, :])
            nc.vector.tensor_copy(otsb[DH:128, pnb * NB:(pnb + 1) * NB],
                                  pacc[1][0:DH, :])
            for j in range(2):
                rr4[j] = rrp.tile([128, 4], dt.float32, tag=f"rr4{j}",
                                  name="rr4")
                nc.sync.dma_start(
                    rr4[j][:],
                    dss[j][:].rearrange("a (p g) -> (a p) g", p=128))
            for q in range(tpb):
                emit_proj_gt(pnb, q, (rr4[0], rr4[1]), jps, tail=True)

        with tc.tile_pool(name="xa", bufs=ntc) as xpool, \
             tc.tile_pool(name="stg", bufs=3) as stgp:
            # k-sliced upfront DMA issue for ALL of x in 1024-token slices
            # (2KB per partition row): the DMA engines stream continuously and
            # the first prep matmul only needs one slice.
            xtiles = []
            for cp in range(ntc // 2):
                for half in range(2):
                    xa = xpool.tile([128, cch, 1024], dt.bfloat16, tag="x")
                    base = half * n_seq + cp * 1024
                    for k in range(cch):
                        nc.sync.dma_start(
                            xa[:, k, :],
                            xt[k * 128:(k + 1) * 128, base:base + 1024])
                    xtiles.append(xa)
                if cp == 0:
                    # zero the padding quadrants of qzt once (DVE is idle now)
                    nc.vector.memset(qzt[:], 0.0)
                if cp == 1:
                    # wp not needed until the first projection (block 1)
                    nc.sync.dma_start(wp_sb[:], wp2)
            xab = []
            for c in range(ntc):
                for half in range(2):
                    xab.append(xtiles[(c // 2) * 2 + half][
                        :, :, (c % 2) * 512:(c % 2) * 512 + 512])
            pools = (xab, stgp)
            # PE warm-up during the initial DMA wait: ~3.5us of matmuls on
            # uninitialized SBUF (no input dependency; results overwritten by
            # the first real accumulations) ramps the pstate/HAM to full rate
            # before the first prep matmul retires.
            wrm = stgp.tile([128, 512], dt.bfloat16, tag="wrm")
            nonlocal_wrm[0] = wrm
            nc.gpsimd.memset(wrm[:], 0.0)
            for w in range(8):
                ps = sps.tile([128, 1024], dt.float32, tag="s", name="warm")
                for k in range(2):
                    nc.tensor.matmul(ps[:, 0:512], wrm[:, 0:128], wrm[:],
                                     start=(k == 0), stop=(k == 1))
            for c in range(3):
                emit_prep(c, *pools)
            sched = {1 + 4 * i: (3 + i,) + pools for i in range(ntc - 3)}
            prev, leftover = emit_block(0, None, None, None, prep_sched=sched)
        for nb in range(1, nblk):
            prev, leftover = emit_block(nb, prev, leftover, sps,
                                        last=(nb == nblk - 1))
        emit_final(prev, sps)
        sps_cm.__exit__(None, None, None)

    persist.release()
    const.release()


def build_kernel(n_seq=N):
    nc = bacc.Bacc("TRN2", target_bir_lowering=False, debug=False, num_devices=8)
    dt = mybir.dt
    t_tot = 2 * n_seq
    xt = nc.dram_tensor("xt", [C, t_tot], dt.bfloat16, kind="ExternalInput").ap()
    wqk = nc.dram_tensor("wqk", [C, 128], dt.bfloat16, kind="ExternalInput").ap()
    wv = nc.dram_tensor("wv", [C, DH], dt.bfloat16, kind="ExternalInput").ap()
    wp2 = nc.dram_tensor("wp2", [128, 2 * C], dt.bfloat16,
                         kind="ExternalInput").ap()
    out = nc.dram_tensor("out", [t_tot, C], dt.bfloat16, kind="ExternalOutput").ap()
    with tile.TileContext(nc) as tc:
        _attention_body(nc, tc, xt, wqk, wv, wp2, out, n_seq)
    nc.compile()
    return nc


def make_in_maps(x, w_qkv, w_proj, n_seq=N):
    """Slice the full inputs into 8 per-core input maps (head per core)."""
    t_tot = 2 * n_seq
    xt = np.ascontiguousarray(x.reshape(t_tot, C).T).astype(BF16)
    in_maps = []
    for h in range(H):
        wq = w_qkv[h * DH:(h + 1) * DH, :].T                      # [C, DH]
        wk = w_qkv[C + h * DH:C + (h + 1) * DH, :].T
        wqk = np.ascontiguousarray(
            np.concatenate([wq, wk], axis=1)).astype(BF16)        # [C, 128]
        wv = np.ascontiguousarray(
            w_qkv[2 * C + h * DH:2 * C + (h + 1) * DH, :].T).astype(BF16)
        wp = np.ascontiguousarray(w_proj[:, h * DH:(h + 1) * DH].T)  # [DH, C]
        wp2 = np.zeros((128, 2 * C), np.float32)   # block-diag [wp 0; 0 wp]
        wp2[0:DH, 0:C] = wp
        wp2[DH:128, C:2 * C] = wp
        wp2 = wp2.astype(BF16)
        in_maps.append({"xt": xt, "wqk": wqk, "wv": wv, "wp2": wp2})
    return in_maps


_NC_CACHE = {}


def _get_nc(n_seq=N):
    if n_seq not in _NC_CACHE:
        _NC_CACHE[n_seq] = build_kernel(n_seq)
    return _NC_CACHE[n_seq]


def run(x, w_qkv, w_proj, b_proj, trace=False, tmpdir=None):
    x = np.asarray(x, dtype=np.float32)
    w_qkv = np.asarray(w_qkv, dtype=np.float32)
    w_proj = np.asarray(w_proj, dtype=np.float32)
    b_proj = np.asarray(b_proj, dtype=np.float32)
    nc = _get_nc()
    in_maps = make_in_maps(x, w_qkv, w_proj)
    try:
        res = run_bass_kernel_spmd(nc, in_maps, list(range(H)), trace=trace,
                                   tmpdir=tmpdir)
    except ModuleNotFoundError:
        res = run_bass_kernel_spmd(nc, in_maps, list(range(H)), trace=False,
                                   tmpdir=tmpdir)
    partial_sum = np.zeros((T, C), np.float32)
    for r in res.results:
        partial_sum += r["out"].astype(np.float32)
    full = partial_sum + b_proj[None, :]
    return full.reshape(B, N, C), res


def kernel(x, w_qkv, w_proj, b_proj):
    out, _ = run(x, w_qkv, w_proj, b_proj)
    return out
